# revision 38
# baseline (speedup 1.0000x reference)
"""Trainium2 Bass kernel: BoxSeg DynamicMaskHead compute_pairwise_term.

For each instance n and each of the 8 non-center taps (dy, dx) of a 3x3
dilation-2 stencil:

    out[n, t, h, w] = sp(x[h,w]) + sp(x[h+dy,w+dx]) - sp(x[h,w] + x[h+dy,w+dx])

with sp = softplus, computed as E = exp(x), L = ln(E + 1) and the tap term
ln(1 + E_c * E_y).  Mirror symmetry out[(dy,dx)][h,w] == out[(-dy,-dx)][h+dy,
w+dx] means only 4 of the 8 tap fields are computed; each is DMA'd to two
output locations.

Row-pair layout: partition p holds image rows {2p, 2p+1} (j = r % 2), so the
whole 256-row frame fits one tile and the dy=-2 row shift is a single
partition shift.  That shift is materialized two ways: for E by an SBUF->SBUF
partition-shift DMA (E2), and for the softplus sum Lsum = L_c + L_y on the
TensorEngine by accumulating two identity matmuls, one with a k=1-shifted
identity (all in bf16: 4x the fp32 matmul rate and fast weight loads).
P = E_c * E_y runs on DVE (one quarter on GpSimd); ln(1+P) on ACT; the final
(ln_tap * -1) + Lsum on DVE reading PSUM.

Intermediates and the DRAM output are bf16 (harness tolerance is 2e-2; bf16
keeps rel err ~1e-2) which halves the dominant HBM write traffic; kernel()
upcasts to float32 on the host.  Each 260-wide row window carries 2-col zero
halos so every output DMA writes full 256-col 128-partition row windows
(multiples of 16 partitions -- the HWDGE only spreads a DMA across the 16
SDMA engines when the partition count divides evenly; a 127-partition DMA
lands on ONE engine at ~22 GB/s).  Out-of-bounds strips are exact zeros
(E2 partition 0 pinned to 1.0 makes ln_tap == Lsum there), memset in SBUF,
or never written (the runner pre-zeros the output buffer); dy=-2 mirror
stores pad down one row-pair into structurally-zero rows of the previous
tap (the one real-data overlap, t5 into t4, is ordered on a single FIFO
queue so t4's rows win).  The work is software-pipelined in groups of G=2
instances with emission order mid(g) -> front(g+1) -> stores(g) so no DMA
queue blocks the next group's critical path.

Sharding: data-parallel over N=64 -> 8 instances per core on 8 NeuronCores.
Self-contained: shapes hardcoded.
"""

import os

import numpy as np

N_CORES = 8
N_FULL = 64
N_PER = N_FULL // N_CORES  # 8 instances per core
H = W = 256
G = 2  # instances per group (batches DMA descriptors); 4 groups per core

# quarters computed directly; q order: (-2,-2), (-2,0), (-2,2), (0,2)
# (t_direct, t_mirror, dy, dx) with taps in F.unfold row-major order
QUARTERS = [
    (0, 7, -2, -2),
    (1, 6, -2, 0),
    (2, 5, -2, 2),
    (4, 3, 0, 2),
]

_CACHE = {}


def _force_combined_act_table():
    """Make the table-load inserter see only the one set containing both Exp
    and Ln (all other sets emptied, positions preserved so act_func_set_id
    still indexes the real act_info.json).  Without this the inserter
    alternates between the exp- and ln-anchored sets: one 1.28us
    ACT_TABLE_LOAD per Exp<->Ln transition, which dominates the runtime."""
    import concourse.bacc as bacc
    import concourse.hw_specs as hw_specs
    import concourse.mybir as mybir

    real = dict(hw_specs.get_activation_tables("gen3"))
    target = None
    for name, fns in real.items():
        if (
            mybir.ActivationFunctionType.Exp in fns
            and mybir.ActivationFunctionType.Ln in fns
        ):
            target = name
            break
    assert target is not None, "no act table set with both Exp and Ln"
    patched = {
        name: (fns if name == target else set()) for name, fns in real.items()
    }
    bacc.get_activation_tables = lambda arch: patched
    hw_specs.get_activation_tables = lambda arch: patched


def _build_program():
    import concourse.bacc as bacc
    import concourse.mybir as mybir
    from concourse import tile

    if not os.environ.get("KERNEL_NO_ACT_PATCH"):
        _force_combined_act_table()

    f32 = mybir.dt.float32
    bf16 = mybir.dt.bfloat16
    EXP = mybir.ActivationFunctionType.Exp
    LN = mybir.ActivationFunctionType.Ln
    ADD = mybir.AluOpType.add
    MULT = mybir.AluOpType.mult

    def mk(base, dims, off=0):
        """Rebuild the free dims of an AP: keep base's partition dim (ap[0]),
        replace the rest with `dims` ([step, count] in elements), and advance
        the offset by `off` elements."""
        c = base.copy()
        c.ap = mybir.VecI64Pair([list(c.ap[0])] + [list(d) for d in dims])
        c.offset = c.offset + off
        return c

    def mkd(base, dims, off=0):
        """Same for DRAM APs (no partition dim to preserve)."""
        c = base.copy()
        c.ap = mybir.VecI64Pair([list(d) for d in dims])
        c.offset = c.offset + off
        return c

    nc = bacc.Bacc(
        "TRN2",
        target_bir_lowering=False,
        debug=False,
        enable_asserts=False,
        num_devices=N_CORES,
    )
    x = nc.dram_tensor("x", [N_PER, H, W], f32, kind="ExternalInput").ap()
    out = nc.dram_tensor("out", [N_PER, 8, H, W], bf16, kind="ExternalOutput").ap()
    eye = nc.dram_tensor("eye", [128, 128], bf16, kind="ExternalInput").ap()
    eye_s1 = nc.dram_tensor("eye_s1", [128, 128], bf16, kind="ExternalInput").ap()

    # element strides in DRAM
    XN = H * W                       # x[n, r, c]
    ON, OT = 8 * H * W, H * W        # out[n, t, r, c]

    # SBUF free-dim layouts (elements per partition); partition p = rows
    # {2p, 2p+1}, j = r % 2, cc = image col + 2 (2-col zero halo each side).
    # X/E/E2/L: [j(2), g(G), cc(260)]
    XJ, XG = G * 260, 260
    XF = 2 * G * 260
    # P/ln_t: [q(4), j(2), g(G), c(256)] (no halo)
    PQ, PJ, PG = 2 * G * 256, G * 256, 256
    PF = 4 * 2 * G * 256
    # o: one tile per wave of W4=4 instances: [q(4), j(2), gg(4), cc(260)]
    W4 = 2 * G
    OQ, OJ, OG = 2 * W4 * 260, W4 * 260, 260
    OF = 4 * 2 * W4 * 260

    with tile.TileContext(nc) as tc:
        with (
            tc.tile_pool(name="cst", bufs=1) as cst,
            tc.tile_pool(name="io", bufs=4) as iop,
            tc.tile_pool(name="wk", bufs=4) as wp,
            tc.tile_pool(name="ps", bufs=4, space="PSUM") as psp,
        ):
            eyet = cst.tile([128, 128], bf16)
            nc.sync.dma_start(out=eyet[:, :], in_=eye[:, :])
            eyes1t = cst.tile([128, 128], bf16)
            nc.scalar.dma_start(out=eyes1t[:, :], in_=eye_s1[:, :])

            NGRP = N_PER // G
            # persistent E2 buffers (round-robin): partition 0 stays 1.0
            # forever (copies only touch partitions 1..127), so the
            # shifted quarters' partition-0 outputs are exactly 0
            e2bufs = []
            for bi in range(3):
                t = cst.tile([128, XF], bf16, tag=f"e2_{bi}")
                nc.vector.memset(mk(t[0:1, 0:1], [[1, XF]]), 1.0)
                e2bufs.append(t)
            # one persistent o buffer per group: halo cols zeroed once
            # up-front (never touched by the combines), so stores do not
            # wait on a per-group halo memset
            obufs = []
            for bi in range(N_PER // W4):
                t = cst.tile([128, OF], bf16, tag=f"o_{bi}")
                nc.vector.memset(
                    mk(t[:, 0:1], [[260, 8 * W4], [258, 2], [1, 2]]), 0.0
                )
                obufs.append(t)

            def front(grp):
                """Input load + Exp/Ln + shifted-E copy for one group.
                Emitted BEFORE the previous group's stores so no queue
                blocks the next group's critical path."""
                n0 = grp * G
                # packed input: partition p <- rows {2p, 2p+1} as one
                # contiguous 512-elem run per instance (2KB descriptors,
                # one trigger per group)
                X = iop.tile([128, G * 512], f32, tag="X")
                eng_in = nc.sync if grp % 2 == 0 else nc.scalar
                eng_in.dma_start(
                    out=mk(X[:, 0:1], [[512, G], [1, 512]]),
                    in_=mkd(x[0, 0:128, :], [[512, 128], [XN, G], [1, 512]],
                            n0 * XN),
                )
                # Exp unpacks [g, (j,c)] -> the halo'd [j, g, cc] layout;
                # the halo cols just get a finite placeholder (0)
                E = iop.tile([128, XF], bf16, tag="E")
                nc.vector.memset(
                    mk(E[:, 0:1], [[260, 2 * G], [258, 2], [1, 2]]), 0.0
                )
                nc.scalar.activation(
                    mk(E[:, 0:1], [[260, G], [XJ, 2], [1, 256]], 2),
                    mk(X[:, 0:1], [[512, G], [256, 2], [1, 256]]), EXP,
                )
                L = iop.tile([128, XF], bf16, tag="L")
                nc.scalar.activation(L[:, :], E[:, :], LN, bias=1.0)
                # E2[p] = E[p-1]: the dy=-2 row shift (rows 2p+j-2), split
                # 112+15 partitions for SDMA-engine spread.  E2[0] = 1.0
                # makes partition 0 of the shifted quarters compute exactly
                # 0 (ln_t == Lsum there), so all the stores can cover the
                # full 128 partitions (a multiple of 16 -- the HWDGE only
                # spreads a DMA across the 16 SDMA engines when the
                # partition count divides evenly).
                E2 = e2bufs[grp % 3]
                nc.gpsimd.dma_start(
                    out=mk(E2[1:113, 0:1], [[1, XF]]),
                    in_=mk(E[0:112, 0:1], [[1, XF]]),
                )
                nc.gpsimd.dma_start(
                    out=mk(E2[113:128, 0:1], [[1, XF]]),
                    in_=mk(E[112:127, 0:1], [[1, XF]]),
                )
                return E, E2, L

            def mid(grp, E, E2, L):
                """P products, ln(1+P), Lsum matmuls, combine into o."""
                # P[q,j,g,c] = E_c * E_y; q0..q2 need the row shift (in1 =
                # E2 at col bases 0,2,4), q3 is col-only (E at base 4).
                # q0..q2 on DVE, q3 on GpSimd.
                P = wp.tile([128, PF], bf16, tag="P")
                for g in range(G):
                    nc.vector.tensor_mul(
                        out=mk(P[:, 0:1], [[PQ, 3], [PJ, 2], [1, 256]], g * PG),
                        in0=mk(E[:, 0:1], [[0, 3], [XJ, 2], [1, 256]], g * XG + 2),
                        in1=mk(E2[:, 0:1], [[2, 3], [XJ, 2], [1, 256]], g * XG),
                    )
                    eng_tt2 = nc.gpsimd if g % 2 else nc.vector
                    eng_tt2.tensor_mul(
                        out=mk(P[:, 0:1], [[PJ, 2], [1, 256]], 3 * PQ + g * PG),
                        in0=mk(E[:, 0:1], [[XJ, 2], [1, 256]], g * XG + 2),
                        in1=mk(E[:, 0:1], [[XJ, 2], [1, 256]], g * XG + 4),
                    )

                ln_t = wp.tile([128, PF], bf16, tag="ln")
                o = obufs[grp // 2]
                gg0 = (grp % 2) * G

                for h in range(2):
                    for g in range(G):
                        # ln(1+P) for this half+instance only, so each
                        # instance's combine unblocks independently
                        nc.scalar.activation(
                            mk(ln_t[:, 0:1], [[PQ, 2], [PJ, 2], [1, 256]],
                               2 * h * PQ + g * PG),
                            mk(P[:, 0:1], [[PQ, 2], [PJ, 2], [1, 256]],
                               2 * h * PQ + g * PG), LN, bias=1.0,
                        )
                        # Lsum = L_c + L_y on the PE: two accumulating
                        # passes into a 2-bank PSUM tile [qh(512), j, c]
                        ps = psp.tile([128, 1024], f32, tag="ps")
                        nc.tensor.matmul(
                            ps[:, 0:512], eyet[:, :],
                            mk(L[:, 0:1], [[XJ, 2], [1, 256]], g * XG + 2),
                            start=True, stop=False,
                        )
                        nc.tensor.matmul(
                            ps[:, 512:1024], eyet[:, :],
                            mk(L[:, 0:1], [[XJ, 2], [1, 256]], g * XG + 2),
                            start=True, stop=(h == 0),
                        )
                        if h == 0:
                            # pass 2: L_y rows-2 via shifted identity,
                            # col bases 0 (dx=-2) and 2 (dx=0)
                            nc.tensor.matmul(
                                ps[:, 0:512], eyes1t[:, :],
                                mk(L[:, 0:1], [[XJ, 2], [1, 256]], g * XG),
                                start=False, stop=True,
                            )
                            nc.tensor.matmul(
                                ps[:, 512:1024], eyes1t[:, :],
                                mk(L[:, 0:1], [[XJ, 2], [1, 256]], g * XG + 2),
                                start=False, stop=True,
                            )
                        else:
                            # q3 (0,+2): same rows, col base 4 (identity,
                            # ordered before the eye_s1 load for q2)
                            nc.tensor.matmul(
                                ps[:, 512:1024], eyet[:, :],
                                mk(L[:, 0:1], [[XJ, 2], [1, 256]], g * XG + 4),
                                start=False, stop=True,
                            )
                            # q2 (-2,+2): rows-2, col base 4
                            nc.tensor.matmul(
                                ps[:, 0:512], eyes1t[:, :],
                                mk(L[:, 0:1], [[XJ, 2], [1, 256]], g * XG + 4),
                                start=False, stop=True,
                            )

                        # o = (ln_t * -1) + Lsum, into the halo'd layout
                        nc.vector.scalar_tensor_tensor(
                            out=mk(o[:, 0:1], [[OQ, 2], [OJ, 2], [1, 256]],
                                   2 * h * OQ + (gg0 + g) * OG + 2),
                            in0=mk(ln_t[:, 0:1], [[PQ, 2], [PJ, 2], [1, 256]],
                                   2 * h * PQ + g * PG),
                            scalar=-1.0,
                            in1=mk(ps[:, 0:1], [[512, 2], [256, 2], [1, 256]]),
                            op0=MULT, op1=ADD,
                        )
                    # out-of-bounds col strips the mirror reads never touch:
                    # q0 (dx=-2): out cols 0,1; q2/q3 (dx=+2): cols 254,255
                    if h == 0:
                        nc.vector.memset(
                            mk(o[:, 0:1], [[OJ, 2], [260, G], [1, 2]],
                               gg0 * OG + 2), 0.0
                        )
                    else:
                        for q in (2, 3):
                            nc.vector.memset(
                                mk(o[:, 0:1], [[OJ, 2], [260, G], [1, 2]],
                                   q * OQ + gg0 * OG + 256), 0.0
                            )
                return o

            def stores(grp, o, last):
                """Each quarter written twice (direct tap + mirror), full
                256-col 128-partition DMAs.  Direct taps: partition 0 of
                the dy=-2 quarters is exact zeros landing on the correct
                rows 0,1 zeros.  dy=-2 mirrors pad DOWN: their partition-0
                zeros land on rows 254,255 of tap tm-1 -- structurally
                zero for t6, t5; for t4 (real data) the t5 store is
                emitted BEFORE the t4 store on the same queue and overlaps
                it in DRAM, so the tracked WAW dependency orders the real
                rows after the padding.  Groups 0,1 store per-group (fast
                pipe fill); the second wave batches all 4 instances per
                trigger (fewer triggers in the drain tail)."""
                gg0 = (grp % 2) * G
                nb = grp * G - gg0  # first instance held in this o tile

                def direct(eng, q, t, j, g0, ng):
                    eng.dma_start(
                        out=mkd(out[0, 0, 0:1, 0:1],
                                [[512, 128], [ON, ng], [1, 256]],
                                (nb + g0) * ON + t * OT + j * 256),
                        in_=mk(o[:, 0:1], [[OG, ng], [1, 256]],
                               q * OQ + j * OJ + g0 * OG + 2),
                    )

                def mirror(eng, q, tm, dy, dx, j, g0, ng):
                    roff = -512 if dy == -2 else 0
                    eng.dma_start(
                        out=mkd(out[0, 0, 0:1, 0:1],
                                [[512, 128], [ON, ng], [1, 256]],
                                (nb + g0) * ON + tm * OT + roff + j * 256),
                        in_=mk(o[:, 0:1], [[OG, ng], [1, 256]],
                               q * OQ + j * OJ + g0 * OG + 2 - dx),
                    )

                if grp == 2:
                    return  # deferred into the wave store below
                g0, ng = (0, W4) if last else (gg0, G)
                for j in range(2):
                    if not last:
                        # A-half streams first (ready earlier); t5 emitted
                        # before the q3/t4 direct on the same FIFO queue
                        direct(nc.sync, 0, 0, j, g0, ng)
                        direct(nc.sync, 1, 1, j, g0, ng)
                        mirror(nc.gpsimd, 0, 7, -2, -2, j, g0, ng)
                        mirror(nc.gpsimd, 1, 6, -2, 0, j, g0, ng)
                        mirror(nc.sync, 2, 5, -2, 2, j, g0, ng)
                        direct(nc.sync, 2, 2, j, g0, ng)
                        direct(nc.sync, 3, 4, j, g0, ng)
                        mirror(nc.gpsimd, 3, 3, 0, 2, j, g0, ng)
                    else:
                        # wave store: scalar's ACT is done, spread wide
                        direct(nc.scalar, 0, 0, j, g0, ng)
                        mirror(nc.gpsimd, 0, 7, -2, -2, j, g0, ng)
                        direct(nc.gpsimd, 1, 1, j, g0, ng)
                        mirror(nc.scalar, 1, 6, -2, 0, j, g0, ng)
                        mirror(nc.sync, 2, 5, -2, 2, j, g0, ng)
                        direct(nc.scalar, 2, 2, j, g0, ng)
                        direct(nc.sync, 3, 4, j, g0, ng)
                        mirror(nc.sync, 3, 3, 0, 2, j, g0, ng)

            cur = front(0)
            for grp in range(NGRP):
                o = mid(grp, *cur)
                if grp + 1 < NGRP:
                    cur = front(grp + 1)
                stores(grp, o, last=(grp + 1 == NGRP))
    nc.compile()
    return nc


def _get_program():
    if "nc" not in _CACHE:
        _CACHE["nc"] = _build_program()
    return _CACHE["nc"]


def _in_maps(xf):
    """Per-core input dicts for run_bass_kernel_spmd from full [64,256,256]."""
    import ml_dtypes

    eye = np.eye(128).astype(ml_dtypes.bfloat16)
    eye_s1 = np.eye(128, k=1).astype(ml_dtypes.bfloat16)
    return [
        {
            "x": np.ascontiguousarray(xf[c * N_PER : (c + 1) * N_PER]),
            "eye": eye,
            "eye_s1": eye_s1,
        }
        for c in range(N_CORES)
    ]


def kernel(mask_logits, pairwise_size=3, pairwise_dilation=2, **_unused):
    assert int(pairwise_size) == 3 and int(pairwise_dilation) == 2
    from concourse.bass_utils import run_bass_kernel_spmd

    xf = np.ascontiguousarray(
        np.asarray(mask_logits, dtype=np.float32).reshape(N_FULL, H, W)
    )
    nc = _get_program()
    res = run_bass_kernel_spmd(nc, _in_maps(xf), core_ids=list(range(N_CORES)))
    return np.concatenate(
        [res.results[c]["out"].astype(np.float32) for c in range(N_CORES)],
        axis=0,
    )


# revision 39
# speedup vs baseline: 1.0446x; 1.0446x over previous
"""Trainium2 Bass kernel: BoxSeg DynamicMaskHead compute_pairwise_term.

For each instance n and each of the 8 non-center taps (dy, dx) of a 3x3
dilation-2 stencil:

    out[n, t, h, w] = sp(x[h,w]) + sp(x[h+dy,w+dx]) - sp(x[h,w] + x[h+dy,w+dx])

with sp = softplus, computed as E = exp(x), L = ln(E + 1) and the tap term
ln(1 + E_c * E_y).  Mirror symmetry out[(dy,dx)][h,w] == out[(-dy,-dx)][h+dy,
w+dx] means only 4 of the 8 tap fields are computed; each is DMA'd to two
output locations.

Row-pair layout: partition p holds image rows {2p, 2p+1} (j = r % 2), so the
whole 256-row frame fits one tile and the dy=-2 row shift is a single
partition shift.  That shift is materialized two ways: for E by an SBUF->SBUF
partition-shift DMA (E2), and for the softplus sum Lsum = L_c + L_y on the
TensorEngine by accumulating two identity matmuls, one with a k=1-shifted
identity (all in bf16: 4x the fp32 matmul rate and fast weight loads).
P = E_c * E_y runs on DVE (one quarter on GpSimd); ln(1+P) on ACT; the final
(ln_tap * -1) + Lsum on DVE reading PSUM.

Intermediates and the DRAM output are bf16 (harness tolerance is 2e-2; bf16
keeps rel err ~1e-2) which halves the dominant HBM write traffic; kernel()
upcasts to float32 on the host.  Each 260-wide row window carries 2-col zero
halos so every output DMA writes full 256-col 128-partition row windows
(multiples of 16 partitions -- the HWDGE only spreads a DMA across the 16
SDMA engines when the partition count divides evenly; a 127-partition DMA
lands on ONE engine at ~22 GB/s).  Out-of-bounds strips are exact zeros
(E2 partition 0 pinned to 1.0 makes ln_tap == Lsum there), memset in SBUF,
or never written (the runner pre-zeros the output buffer); dy=-2 mirror
stores pad down one row-pair into structurally-zero rows of the previous
tap (the one real-data overlap, t5 into t4, is ordered on a single FIFO
queue so t4's rows win).  The work is software-pipelined in groups of G=2
instances with emission order mid(g) -> front(g+1) -> stores(g) so no DMA
queue blocks the next group's critical path.

Sharding: data-parallel over N=64 -> 8 instances per core on 8 NeuronCores.
Self-contained: shapes hardcoded.
"""

import os

import numpy as np

N_CORES = 8
N_FULL = 64
N_PER = N_FULL // N_CORES  # 8 instances per core
H = W = 256
G = 2  # instances per group (batches DMA descriptors); 4 groups per core

# quarters computed directly; q order: (-2,-2), (-2,0), (-2,2), (0,2)
# (t_direct, t_mirror, dy, dx) with taps in F.unfold row-major order
QUARTERS = [
    (0, 7, -2, -2),
    (1, 6, -2, 0),
    (2, 5, -2, 2),
    (4, 3, 0, 2),
]

_CACHE = {}


def _force_combined_act_table():
    """Make the table-load inserter see only the one set containing both Exp
    and Ln (all other sets emptied, positions preserved so act_func_set_id
    still indexes the real act_info.json).  Without this the inserter
    alternates between the exp- and ln-anchored sets: one 1.28us
    ACT_TABLE_LOAD per Exp<->Ln transition, which dominates the runtime."""
    import concourse.bacc as bacc
    import concourse.hw_specs as hw_specs
    import concourse.mybir as mybir

    real = dict(hw_specs.get_activation_tables("gen3"))
    target = None
    for name, fns in real.items():
        if (
            mybir.ActivationFunctionType.Exp in fns
            and mybir.ActivationFunctionType.Ln in fns
        ):
            target = name
            break
    assert target is not None, "no act table set with both Exp and Ln"
    patched = {
        name: (fns if name == target else set()) for name, fns in real.items()
    }
    bacc.get_activation_tables = lambda arch: patched
    hw_specs.get_activation_tables = lambda arch: patched


def _build_program():
    import concourse.bacc as bacc
    import concourse.mybir as mybir
    from concourse import tile

    if not os.environ.get("KERNEL_NO_ACT_PATCH"):
        _force_combined_act_table()

    f32 = mybir.dt.float32
    bf16 = mybir.dt.bfloat16
    EXP = mybir.ActivationFunctionType.Exp
    LN = mybir.ActivationFunctionType.Ln
    ADD = mybir.AluOpType.add
    MULT = mybir.AluOpType.mult

    def mk(base, dims, off=0):
        """Rebuild the free dims of an AP: keep base's partition dim (ap[0]),
        replace the rest with `dims` ([step, count] in elements), and advance
        the offset by `off` elements."""
        c = base.copy()
        c.ap = mybir.VecI64Pair([list(c.ap[0])] + [list(d) for d in dims])
        c.offset = c.offset + off
        return c

    def mkd(base, dims, off=0):
        """Same for DRAM APs (no partition dim to preserve)."""
        c = base.copy()
        c.ap = mybir.VecI64Pair([list(d) for d in dims])
        c.offset = c.offset + off
        return c

    nc = bacc.Bacc(
        "TRN2",
        target_bir_lowering=False,
        debug=False,
        enable_asserts=False,
        num_devices=N_CORES,
    )
    x = nc.dram_tensor("x", [N_PER, H, W], f32, kind="ExternalInput").ap()
    out = nc.dram_tensor("out", [N_PER, 8, H, W], bf16, kind="ExternalOutput").ap()
    eye = nc.dram_tensor("eye", [128, 128], bf16, kind="ExternalInput").ap()
    eye_s1 = nc.dram_tensor("eye_s1", [128, 128], bf16, kind="ExternalInput").ap()

    # element strides in DRAM
    XN = H * W                       # x[n, r, c]
    ON, OT = 8 * H * W, H * W        # out[n, t, r, c]

    # SBUF free-dim layouts (elements per partition); partition p = rows
    # {2p, 2p+1}, j = r % 2, cc = image col + 2 (2-col zero halo each side).
    # X/E/E2/L: [j(2), g(G), cc(260)]
    XJ, XG = G * 260, 260
    XF = 2 * G * 260
    # P/ln_t: [q(4), j(2), g(G), c(256)] (no halo)
    PQ, PJ, PG = 2 * G * 256, G * 256, 256
    PF = 4 * 2 * G * 256
    # o: [q(4), j(2), g(G), cc(260)]
    OQ, OJ, OG = 2 * G * 260, G * 260, 260
    OF = 4 * 2 * G * 260

    with tile.TileContext(nc) as tc:
        with (
            tc.tile_pool(name="cst", bufs=1) as cst,
            tc.tile_pool(name="io", bufs=4) as iop,
            tc.tile_pool(name="wk", bufs=4) as wp,
            tc.tile_pool(name="ps", bufs=4, space="PSUM") as psp,
        ):
            eyet = cst.tile([128, 128], bf16)
            nc.sync.dma_start(out=eyet[:, :], in_=eye[:, :])
            eyes1t = cst.tile([128, 128], bf16)
            nc.scalar.dma_start(out=eyes1t[:, :], in_=eye_s1[:, :])

            NGRP = N_PER // G
            # persistent E2 buffers (round-robin): partition 0 stays 1.0
            # forever (copies only touch partitions 1..127), so the
            # shifted quarters' partition-0 outputs are exactly 0
            e2bufs = []
            for bi in range(3):
                t = cst.tile([128, XF], bf16, tag=f"e2_{bi}")
                nc.vector.memset(mk(t[0:1, 0:1], [[1, XF]]), 1.0)
                e2bufs.append(t)
            # one persistent o buffer per group: halo cols zeroed once
            # up-front (never touched by the combines), so stores do not
            # wait on a per-group halo memset
            obufs = []
            for bi in range(N_PER // G):
                t = cst.tile([128, OF], bf16, tag=f"o_{bi}")
                nc.vector.memset(
                    mk(t[:, 0:1], [[260, 8 * G], [258, 2], [1, 2]]), 0.0
                )
                obufs.append(t)

            def front(grp):
                """Input load + Exp/Ln + shifted-E copy for one group.
                Emitted BEFORE the previous group's stores so no queue
                blocks the next group's critical path."""
                n0 = grp * G
                # packed input: partition p <- rows {2p, 2p+1} as one
                # contiguous 512-elem run per instance (2KB descriptors,
                # one trigger per group)
                X = iop.tile([128, G * 512], f32, tag="X")
                eng_in = nc.sync if grp % 2 == 0 else nc.scalar
                eng_in.dma_start(
                    out=mk(X[:, 0:1], [[512, G], [1, 512]]),
                    in_=mkd(x[0, 0:128, :], [[512, 128], [XN, G], [1, 512]],
                            n0 * XN),
                )
                # Exp unpacks [g, (j,c)] -> the halo'd [j, g, cc] layout;
                # the halo cols just get a finite placeholder (0)
                E = iop.tile([128, XF], bf16, tag="E")
                nc.vector.memset(
                    mk(E[:, 0:1], [[260, 2 * G], [258, 2], [1, 2]]), 0.0
                )
                nc.scalar.activation(
                    mk(E[:, 0:1], [[260, G], [XJ, 2], [1, 256]], 2),
                    mk(X[:, 0:1], [[512, G], [256, 2], [1, 256]]), EXP,
                )
                L = iop.tile([128, XF], bf16, tag="L")
                nc.scalar.activation(L[:, :], E[:, :], LN, bias=1.0)
                # E2[p] = E[p-1]: the dy=-2 row shift (rows 2p+j-2), split
                # 112+15 partitions for SDMA-engine spread.  E2[0] = 1.0
                # makes partition 0 of the shifted quarters compute exactly
                # 0 (ln_t == Lsum there), so all the stores can cover the
                # full 128 partitions (a multiple of 16 -- the HWDGE only
                # spreads a DMA across the 16 SDMA engines when the
                # partition count divides evenly).
                E2 = e2bufs[grp % 3]
                nc.gpsimd.dma_start(
                    out=mk(E2[1:113, 0:1], [[1, XF]]),
                    in_=mk(E[0:112, 0:1], [[1, XF]]),
                )
                nc.gpsimd.dma_start(
                    out=mk(E2[113:128, 0:1], [[1, XF]]),
                    in_=mk(E[112:127, 0:1], [[1, XF]]),
                )
                return E, E2, L

            def mid(grp, E, E2, L):
                """P products, ln(1+P), Lsum matmuls, combine into o."""
                # P[q,j,g,c] = E_c * E_y; q0..q2 need the row shift (in1 =
                # E2 at col bases 0,2,4), q3 is col-only (E at base 4).
                # q0..q2 on DVE, q3 on GpSimd.
                P = wp.tile([128, PF], bf16, tag="P")
                for g in range(G):
                    nc.vector.tensor_mul(
                        out=mk(P[:, 0:1], [[PQ, 3], [PJ, 2], [1, 256]], g * PG),
                        in0=mk(E[:, 0:1], [[0, 3], [XJ, 2], [1, 256]], g * XG + 2),
                        in1=mk(E2[:, 0:1], [[2, 3], [XJ, 2], [1, 256]], g * XG),
                    )
                    eng_tt2 = nc.gpsimd if g % 2 else nc.vector
                    eng_tt2.tensor_mul(
                        out=mk(P[:, 0:1], [[PJ, 2], [1, 256]], 3 * PQ + g * PG),
                        in0=mk(E[:, 0:1], [[XJ, 2], [1, 256]], g * XG + 2),
                        in1=mk(E[:, 0:1], [[XJ, 2], [1, 256]], g * XG + 4),
                    )

                ln_t = wp.tile([128, PF], bf16, tag="ln")
                o = obufs[grp]

                for h in range(2):
                    for g in range(G):
                        # ln(1+P) for this half+instance only, so each
                        # instance's combine unblocks independently
                        nc.scalar.activation(
                            mk(ln_t[:, 0:1], [[PQ, 2], [PJ, 2], [1, 256]],
                               2 * h * PQ + g * PG),
                            mk(P[:, 0:1], [[PQ, 2], [PJ, 2], [1, 256]],
                               2 * h * PQ + g * PG), LN, bias=1.0,
                        )
                        # Lsum = L_c + L_y on the PE: two accumulating
                        # passes into a 2-bank PSUM tile [qh(512), j, c]
                        ps = psp.tile([128, 1024], f32, tag="ps")
                        nc.tensor.matmul(
                            ps[:, 0:512], eyet[:, :],
                            mk(L[:, 0:1], [[XJ, 2], [1, 256]], g * XG + 2),
                            start=True, stop=False,
                        )
                        nc.tensor.matmul(
                            ps[:, 512:1024], eyet[:, :],
                            mk(L[:, 0:1], [[XJ, 2], [1, 256]], g * XG + 2),
                            start=True, stop=(h == 0),
                        )
                        if h == 0:
                            # pass 2: L_y rows-2 via shifted identity,
                            # col bases 0 (dx=-2) and 2 (dx=0)
                            nc.tensor.matmul(
                                ps[:, 0:512], eyes1t[:, :],
                                mk(L[:, 0:1], [[XJ, 2], [1, 256]], g * XG),
                                start=False, stop=True,
                            )
                            nc.tensor.matmul(
                                ps[:, 512:1024], eyes1t[:, :],
                                mk(L[:, 0:1], [[XJ, 2], [1, 256]], g * XG + 2),
                                start=False, stop=True,
                            )
                        else:
                            # q3 (0,+2): same rows, col base 4 (identity,
                            # ordered before the eye_s1 load for q2)
                            nc.tensor.matmul(
                                ps[:, 512:1024], eyet[:, :],
                                mk(L[:, 0:1], [[XJ, 2], [1, 256]], g * XG + 4),
                                start=False, stop=True,
                            )
                            # q2 (-2,+2): rows-2, col base 4
                            nc.tensor.matmul(
                                ps[:, 0:512], eyes1t[:, :],
                                mk(L[:, 0:1], [[XJ, 2], [1, 256]], g * XG + 4),
                                start=False, stop=True,
                            )

                        # o = (ln_t * -1) + Lsum, into the halo'd layout
                        nc.vector.scalar_tensor_tensor(
                            out=mk(o[:, 0:1], [[OQ, 2], [OJ, 2], [1, 256]],
                                   2 * h * OQ + g * OG + 2),
                            in0=mk(ln_t[:, 0:1], [[PQ, 2], [PJ, 2], [1, 256]],
                                   2 * h * PQ + g * PG),
                            scalar=-1.0,
                            in1=mk(ps[:, 0:1], [[512, 2], [256, 2], [1, 256]]),
                            op0=MULT, op1=ADD,
                        )
                    # out-of-bounds col strips the mirror reads never touch:
                    # q0 (dx=-2): out cols 0,1; q2/q3 (dx=+2): cols 254,255
                    if h == 0:
                        nc.vector.memset(
                            mk(o[:, 0:1], [[260, 2 * G], [1, 2]], 2), 0.0
                        )
                    else:
                        nc.vector.memset(
                            mk(o[:, 0:1], [[260, 4 * G], [1, 2]],
                               2 * OQ + 256), 0.0
                        )
                return o

            def stores(grp, o, last):
                """Each quarter written twice (direct tap + mirror), all G
                instances per trigger, full 256-col 128-partition DMAs.
                Direct taps: partition 0 of the dy=-2 quarters is exact
                zeros landing on the correct rows 0,1 zeros.  dy=-2
                mirrors pad DOWN: their partition-0 zeros land on rows
                254,255 of tap tm-1 -- structurally zero for t6, t5; for
                t4 (real data) the t5 store is emitted BEFORE the t4 store
                on the same queue and overlaps it in DRAM, so the tracked
                WAW dependency orders the real rows after the padding."""
                n0 = grp * G

                def direct(eng, q, t, j):
                    eng.dma_start(
                        out=mkd(out[0, 0, 0:1, 0:1],
                                [[512, 128], [ON, G], [1, 256]],
                                n0 * ON + t * OT + j * 256),
                        in_=mk(o[:, 0:1], [[OG, G], [1, 256]],
                               q * OQ + j * OJ + 2),
                    )

                def mirror(eng, q, tm, dy, dx, j):
                    roff = -512 if dy == -2 else 0
                    eng.dma_start(
                        out=mkd(out[0, 0, 0:1, 0:1],
                                [[512, 128], [ON, G], [1, 256]],
                                n0 * ON + tm * OT + roff + j * 256),
                        in_=mk(o[:, 0:1], [[OG, G], [1, 256]],
                               q * OQ + j * OJ + 2 - dx),
                    )

                for j in range(2):
                    if not last:
                        # A-half streams first (ready earlier); t5 emitted
                        # before the q3/t4 direct on the same FIFO queue
                        direct(nc.sync, 0, 0, j)
                        direct(nc.sync, 1, 1, j)
                        mirror(nc.gpsimd, 0, 7, -2, -2, j)
                        mirror(nc.gpsimd, 1, 6, -2, 0, j)
                        mirror(nc.sync, 2, 5, -2, 2, j)
                        direct(nc.sync, 2, 2, j)
                        direct(nc.sync, 3, 4, j)
                        mirror(nc.gpsimd, 3, 3, 0, 2, j)
                    else:
                        # last group: scalar's ACT is done, spread the tail
                        mirror(nc.gpsimd, 0, 7, -2, -2, j)
                        direct(nc.scalar, 0, 0, j)
                        mirror(nc.scalar, 1, 6, -2, 0, j)
                        direct(nc.scalar, 1, 1, j)
                        mirror(nc.sync, 2, 5, -2, 2, j)
                        direct(nc.scalar, 2, 2, j)
                        mirror(nc.sync, 3, 3, 0, 2, j)
                        direct(nc.sync, 3, 4, j)

            cur = front(0)
            for grp in range(NGRP):
                o = mid(grp, *cur)
                if grp + 1 < NGRP:
                    cur = front(grp + 1)
                stores(grp, o, last=(grp + 1 == NGRP))
    nc.compile()
    return nc


def _get_program():
    if "nc" not in _CACHE:
        _CACHE["nc"] = _build_program()
    return _CACHE["nc"]


def _in_maps(xf):
    """Per-core input dicts for run_bass_kernel_spmd from full [64,256,256]."""
    import ml_dtypes

    eye = np.eye(128).astype(ml_dtypes.bfloat16)
    eye_s1 = np.eye(128, k=1).astype(ml_dtypes.bfloat16)
    return [
        {
            "x": np.ascontiguousarray(xf[c * N_PER : (c + 1) * N_PER]),
            "eye": eye,
            "eye_s1": eye_s1,
        }
        for c in range(N_CORES)
    ]


def kernel(mask_logits, pairwise_size=3, pairwise_dilation=2, **_unused):
    assert int(pairwise_size) == 3 and int(pairwise_dilation) == 2
    from concourse.bass_utils import run_bass_kernel_spmd

    xf = np.ascontiguousarray(
        np.asarray(mask_logits, dtype=np.float32).reshape(N_FULL, H, W)
    )
    nc = _get_program()
    res = run_bass_kernel_spmd(nc, _in_maps(xf), core_ids=list(range(N_CORES)))
    return np.concatenate(
        [res.results[c]["out"].astype(np.float32) for c in range(N_CORES)],
        axis=0,
    )
